# revision 1
# baseline (speedup 1.0000x reference)
"""Locally-connected autoencoder (128 independent 256->8->256 per-patch linears
+ sigmoid) on 8 Trainium2 NeuronCores.

Strategy
--------
The per-patch GEMMs contract over the patch dimension, so the patch dim must
sit on SBUF partitions.  Instead of burning PE cycles on on-chip transposes,
the host ships x already transposed: per core [2 bands, 16 rows, 128 w,
2048 samples], where a "band" is one row of 8 patches (16 image rows = 2048
contiguous floats of each sample's row).  Core k owns bands (2k, 2k+1) for
ALL samples, which keeps replicated weight traffic ~1 MiB/core.

fp32 matmuls on TRN2 are ~4x slower than bf16 (2-pass hi/lo emulation at
half stream rate).  We instead use a compensated bf16 split: a = ah + al
with ah = bf16(a), al = bf16(a - ah).  ah.wh products are EXACT in the fp32
PSUM (8x8-bit mantissas), so  a.w ~= ah.wh + ah.wl + al.wh  carries ~2^-18
relative error - 3 bf16 matmuls at ~1/4 the fp32 cost each.  x and the
block-diagonal weights are split on the host (same DMA bytes as fp32 x);
the latent z is split on-chip by DVE.

Per band:
  encode:  z^T[64(pw,h), n512] = sum_r Wenc_bd[r].T @ X[r]  (48 bf16 matmuls
           accumulated in PSUM; Wenc_bd is the block-diag arrangement of We)
  bias:    ACT copies z^T PSUM->SBUF fp32 adding be as per-partition bias;
           DVE splits z into (zh, zl) bf16 with a 65th row = (1, 0) so the
           decode bias rides in wdec row 64.
  decode (transposed): out^T[128 f, n512] = Wdec_bd[:, fchunk].T @ z_aug
           (3 bf16 matmuls per fchunk, K=65); fchunk = one image row r'.
  sigmoid: ACT reads decode PSUM -> SBUF out^T tiles -> DMA to HBM.
The host re-transposes the per-core out^T back to [n, f].
"""

import numpy as np

# problem constants (hardcoded per contract)
H, W, PS = 256, 128, 16
NPH, NPW = H // PS, W // PS      # 16 bands, 8 patches/band
P, D, HID = NPH * NPW, PS * PS, 8
NSMP = 4 * 512                    # 2048 samples
BANDW = PS * W                    # 2048 floats per band per sample
NCORES = 8
BPC = NPH // NCORES               # 2 bands per core
M = NPW * HID                     # 64 latent rows per band
KD = M + 1                        # decode contraction (with bias row)

_PROG = None
LAST_EXEC_NS = None   # filled when kernel() runs with _trace=True


def _install_ntff_hook():
    """The agent image's antenv lacks axon_hooks; synthesize it so
    run_bass_kernel_spmd(trace=True) can capture NTFF profiles."""
    import sys, types
    try:
        import antenv.axon_hooks  # noqa: F401
        return
    except ImportError:
        pass
    try:
        from trn_agent_boot.trn_boot import _ntff_profile_via_ctypes
        hook = _ntff_profile_via_ctypes('/opt/axon/libaxon_pjrt.so')
    except Exception:
        hook = None
    import antenv
    mod = types.ModuleType("antenv.axon_hooks")
    mod.get_axon_ntff_profile_hook = lambda: hook
    mod.set_axon_ntff_profile_hook = lambda h: None
    antenv.axon_hooks = mod
    sys.modules["antenv.axon_hooks"] = mod


def _patch_tile_drain():
    """This image's walrus caps instructions at ONE sync wait.  Tile attaches
    one wait per outstanding semaphore to the exit drain and can give body
    instructions several waits.  Split: hoist all but one wait onto fresh
    single-wait NOPs inserted immediately before, on the same engine (engine
    streams are in-order, so this is semantics-preserving)."""
    import concourse.tile as tile
    import bass_rust
    from concourse.vector_clock import ScopedClock

    if getattr(tile.TileContext, "_drain_split_patched", False):
        return

    def patched(self, tick_clock, wait_clock):
        drain_inst = self.nc.sync.drain()
        wait_clock.add_sem_waits(
            drain_inst.ins, ScopedClock({None: tick_clock.global_clock})
        )
        si = drain_inst.ins.sync_info
        w = si.on_wait if si else []
        if len(w) > 1:
            drain_inst.ins.sync_info.on_wait = w[:1]
            for x in w[1:]:
                d2 = self.nc.sync.drain()
                d2.ins.sync_info = bass_rust.SyncInfo(on_wait=[x], on_update=[])
        self.nc.all_engine_barrier()
        assert self.sems is not None
        popped = self.nc._tile_sem_poison_stack.pop()
        assert popped is self._sem_poison
        self.nc.clear_and_free_semaphores(list(self.sems.allocated().values()))
        self.nc.all_engine_barrier()

    tile.TileContext._drain_and_barrier = patched

    from concourse import mybir
    from concourse.tile_scheduler import BassTileLoopBlock, BassTileRelease

    _special = [BassTileLoopBlock, BassTileRelease]
    for nm in ("BassTileCriticalSection", "BassTileBranchHintPlaceholder",
               "TileBranchInst", "BassTileConditionalBlock"):
        cls = getattr(tile, nm, None)
        if cls is not None:
            _special.append(cls)
    _special = tuple(_special)

    orig_lower = tile.TileContext._lower_ordered_insts

    def patched_lower(self, ordered):
        for bb_name in list(ordered.keys()):
            insts = ordered[bb_name]
            if not any(
                i.sync_info is not None and len(i.sync_info.on_wait) > 1
                for i in insts
            ):
                continue
            new = []
            for inst in insts:
                si = inst.sync_info
                if (
                    si is not None
                    and len(si.on_wait) > 1
                    and not isinstance(inst, _special)
                ):
                    waits = list(si.on_wait)
                    for x in waits[:-1]:
                        nop = mybir.InstNoOp(
                            name=self.nc.get_next_instruction_name(),
                            ins=[],
                            outs=[],
                            engine=inst.engine,
                            bass_nofuse=True,
                            sync_info=bass_rust.SyncInfo(on_wait=[x], on_update=[]),
                        )
                        new.append(nop)
                    si.on_wait = waits[-1:]
                new.append(inst)
            ordered[bb_name] = new
        return orig_lower(self, ordered)

    tile.TileContext._lower_ordered_insts = patched_lower
    tile.TileContext._drain_split_patched = True


def _build_program():
    """Build the per-core Bass program (same program for all 8 cores)."""
    global _PROG
    if _PROG is not None:
        return _PROG

    import concourse.bass as bass
    import concourse.tile as tile
    from concourse import mybir

    _patch_tile_drain()

    f32 = mybir.dt.float32
    bf16 = mybir.dt.bfloat16
    AFT = mybir.ActivationFunctionType
    ADD = mybir.AluOpType.add
    SUB = mybir.AluOpType.subtract

    nc = bass.Bass("TRN2", target_bir_lowering=False, debug=False)

    xth_d = nc.dram_tensor("xth", [BPC, PS, W, NSMP], bf16, kind="ExternalInput").ap()
    xtl_d = nc.dram_tensor("xtl", [BPC, PS, W, NSMP], bf16, kind="ExternalInput").ap()
    # encode stationary: [weh | wel] stacked along the output (m) dim
    we_d = nc.dram_tensor("we2", [BPC, PS, W, 2 * M], bf16, kind="ExternalInput").ap()
    # decode stationary: [wdh ; wdl] stacked along the contraction (K) dim
    wd_d = nc.dram_tensor("wd2", [BPC, 2 * M, BANDW], bf16, kind="ExternalInput").ap()
    bev_d = nc.dram_tensor("bev", [M, BPC], f32, kind="ExternalInput").ap()
    bdv_d = nc.dram_tensor("bdv", [W, BPC, PS], f32, kind="ExternalInput").ap()
    out_d = nc.dram_tensor("out", [BPC, PS, W, NSMP], f32, kind="ExternalOutput").ap()

    with tile.TileContext(nc) as tc:
        with (
            tc.tile_pool(name="singles", bufs=1) as singles,
            tc.tile_pool(name="xhp", bufs=2) as xhpool,
            tc.tile_pool(name="xlp", bufs=2) as xlpool,
            tc.tile_pool(name="zf", bufs=3) as zfpool,
            tc.tile_pool(name="zhp", bufs=6) as zhpool,
            tc.tile_pool(name="zlp", bufs=6) as zlpool,
            tc.tile_pool(name="outsb", bufs=3) as opool,
            tc.tile_pool(name="zps", bufs=2, space="PSUM") as zpsum,
            tc.tile_pool(name="ops", bufs=5, space="PSUM") as opsum,
        ):
            # --- encode weights first: the first matmul needs only these ---
            we_sb = singles.tile([W, BPC, PS, 2 * M], bf16)
            nc.sync.dma_start(out=we_sb, in_=we_d.rearrange("b r w m -> w b r m"))

            # x loads at n1024 granularity (2 MiB each); issue the first
            # chunk before the decode-side weights so compute starts early.
            x_tiles = {}

            def load_x(b, ni):
                n0 = 1024 * ni
                Xh = xhpool.tile([W, PS, 1024], bf16, name="Xh")
                nc.sync.dma_start(
                    out=Xh,
                    in_=xth_d[b, :, :, n0:n0 + 1024].rearrange("r w n -> w r n"),
                )
                Xl = xlpool.tile([W, PS, 1024], bf16, name="Xl")
                nc.sync.dma_start(
                    out=Xl,
                    in_=xtl_d[b, :, :, n0:n0 + 1024].rearrange("r w n -> w r n"),
                )
                x_tiles[(b, ni)] = (Xh, Xl)

            import os
            _reorder = os.environ.get("K_REORDER", "0") == "1"
            _prefetch = os.environ.get("K_PREFETCH", "0") == "1"
            if _reorder:
                load_x(0, 0)

            bev_sb = singles.tile([M, BPC], f32)
            nc.sync.dma_start(out=bev_sb, in_=bev_d)
            bdv_sb = singles.tile([W, BPC, PS], f32)
            nc.sync.dma_start(out=bdv_sb, in_=bdv_d)
            wd_sb = singles.tile([2 * M, BPC, BANDW], bf16)
            nc.sync.dma_start(out=wd_sb, in_=wd_d.rearrange("b m f -> m b f"))

            for b in range(BPC):
                zh_tiles, zl_tiles = [], []
                for ni in range(2):            # n1024 chunks
                    if (b, ni) not in x_tiles:
                        load_x(b, ni)
                    if _prefetch:
                        nxt = (b, ni + 1) if ni < 1 else (b + 1, 0)
                        if nxt[0] < BPC and nxt not in x_tiles:
                            load_x(*nxt)
                    Xh, Xl = x_tiles.pop((b, ni))
                    for nj in range(2):        # n512 chunks
                        s0 = 512 * nj
                        # z_ps rows 0..63 : weh.T @ (Xh+Xl); rows 64..127 : wel.T @ (Xh+Xl)
                        z_ps = zpsum.tile([2 * M, 512], f32, name="z_ps")
                        for r in range(PS):
                            nc.tensor.matmul(
                                z_ps, lhsT=we_sb[:, b, r, :],
                                rhs=Xh[:, r, s0:s0 + 512],
                                start=(r == 0), stop=False,
                            )
                            nc.tensor.matmul(
                                z_ps, lhsT=we_sb[:, b, r, :],
                                rhs=Xl[:, r, s0:s0 + 512],
                                start=False, stop=(r == PS - 1),
                            )
                        # z = top + bottom + be  (ACT folds be, DVE adds halves)
                        t1 = zfpool.tile([M, 512], f32, name="t1")
                        nc.scalar.activation(
                            out=t1, in_=z_ps[M:2 * M, :], func=AFT.Identity,
                            bias=bev_sb[:, b:b + 1], scale=1.0,
                        )
                        z_sb = zfpool.tile([M, 512], f32, name="z_sb")
                        nc.vector.tensor_tensor(z_sb, z_ps[0:M, :], t1, ADD)
                        # split z into bf16 hi/lo, duplicated into both K-halves
                        zh = zhpool.tile([2 * M, 512], bf16, name="zh")
                        nc.vector.tensor_copy(zh[0:M, :], z_sb)
                        nc.vector.tensor_copy(zh[M:2 * M, :], zh[0:M, :])
                        zl = zlpool.tile([2 * M, 512], bf16, name="zl")
                        nc.vector.tensor_tensor(zl[0:M, :], z_sb, zh[0:M, :], SUB)
                        nc.vector.tensor_copy(zl[M:2 * M, :], zl[0:M, :])
                        zh_tiles.append(zh)
                        zl_tiles.append(zl)

                for fc in range(PS):           # one image row r\' per chunk
                    o_sb = opool.tile([W, NSMP], f32, name="o_sb")
                    wstat = wd_sb[:, b, fc * W:(fc + 1) * W]
                    for j in range(4):
                        o_ps = opsum.tile([W, 512], f32, name="o_ps")
                        nc.tensor.matmul(o_ps, lhsT=wstat, rhs=zh_tiles[j],
                                         start=True, stop=False)
                        nc.tensor.matmul(o_ps, lhsT=wstat, rhs=zl_tiles[j],
                                         start=False, stop=True)
                        nc.scalar.activation(
                            out=o_sb[:, 512 * j:512 * (j + 1)],
                            in_=o_ps, func=AFT.Sigmoid,
                            bias=bdv_sb[:, b, fc:fc + 1], scale=1.0,
                        )
                    nc.scalar.dma_start(out=out_d[b, fc], in_=o_sb)

    _PROG = nc
    return nc


def _bf16_split(a):
    import ml_dtypes
    hi = a.astype(ml_dtypes.bfloat16)
    lo = (a - hi.astype(np.float32)).astype(ml_dtypes.bfloat16)
    return hi, lo


def _host_prep(x, We, be, Wd, bd):
    """Slice/transpose/split inputs into per-core maps (pure numpy)."""
    x = np.ascontiguousarray(np.asarray(x, dtype=np.float32)).reshape(NSMP, H * W)
    We = np.asarray(We, dtype=np.float32)
    be = np.asarray(be, dtype=np.float32)
    Wd = np.asarray(Wd, dtype=np.float32)
    bd = np.asarray(bd, dtype=np.float32)

    xT = np.ascontiguousarray(x.T)                       # [32768, 2048]

    # encode block-diag: wenc[ph, r, 16pw+c, 8pw+h] = We[ph*8+pw, h, r*16+c]
    We6 = We.reshape(NPH, NPW, HID, PS, PS)              # [ph, pw, h, r, c]
    wenc = np.zeros((NPH, PS, W, M), dtype=np.float32)
    for pw in range(NPW):
        wenc[:, :, PS * pw:PS * (pw + 1), HID * pw:HID * (pw + 1)] = (
            We6[:, pw].transpose(0, 2, 3, 1)             # [ph, r, c, h]
        )
    weh, wel = _bf16_split(wenc)
    we2 = np.concatenate([weh, wel], axis=-1)            # [ph, r, w, 128]

    # decode rhs: wdec[ph, 8pw+h, 128r\'+16pw+c\'] = Wd[ph*8+pw, r\'*16+c\', h]
    Wd5 = Wd.reshape(NPH, NPW, PS, PS, HID)              # [ph, pw, r\', c\', h]
    wdec = np.zeros((NPH, M, BANDW), dtype=np.float32)
    wdec_v = wdec.reshape(NPH, NPW, HID, PS, NPW, PS)
    for pw in range(NPW):
        wdec_v[:, pw, :, :, pw, :] = Wd5[:, pw].transpose(0, 3, 1, 2)  # [ph, h, r\', c\']
    wdh, wdl = _bf16_split(wdec)
    wd2 = np.concatenate([wdh, wdl], axis=1)             # [ph, 128, BANDW]

    bev = be.reshape(NPH, M)                             # [ph, 64]
    # decode bias, per-partition for the transposed output: bdv[w\', ph, r\']
    bd4 = bd.reshape(NPH, NPW, PS, PS)                   # [ph, pw, r\', c\']
    bdv = bd4.transpose(1, 3, 0, 2).reshape(W, NPH, PS)  # [16pw+c\', ph, r\']

    in_maps = []
    for k in range(NCORES):
        xt_k = xT[BPC * BANDW * k: BPC * BANDW * (k + 1)].reshape(BPC, PS, W, NSMP)
        xth, xtl = _bf16_split(xt_k)
        in_maps.append({
            "xth": np.ascontiguousarray(xth),
            "xtl": np.ascontiguousarray(xtl),
            "we2": np.ascontiguousarray(we2[BPC * k:BPC * (k + 1)]),
            "wd2": np.ascontiguousarray(wd2[BPC * k:BPC * (k + 1)]),
            "bev": np.ascontiguousarray(bev[BPC * k:BPC * (k + 1)].T),
            "bdv": np.ascontiguousarray(bdv[:, BPC * k:BPC * (k + 1), :]),
        })
    return in_maps


def kernel(x, We, be, Wd, bd, _trace=False):
    global LAST_EXEC_NS
    from concourse.bass_utils import run_bass_kernel_spmd

    if _trace:
        _install_ntff_hook()

    nc = _build_program()
    in_maps = _host_prep(x, We, be, Wd, bd)
    res = run_bass_kernel_spmd(nc, in_maps, list(range(NCORES)), trace=_trace)
    if _trace:
        LAST_EXEC_NS = res.exec_time_ns

    # out_k is out^T: [band, r'(=fchunk), w', n]  ->  out[n, band*2048 + 128 r' + w']
    cols = [
        res.results[k]["out"].reshape(BPC * BANDW, NSMP).T
        for k in range(NCORES)
    ]
    out = np.concatenate(cols, axis=1)
    return np.ascontiguousarray(out.reshape(4, 512, H * W), dtype=np.float32)



# revision 4
# speedup vs baseline: 1.6131x; 1.6131x over previous
"""Locally-connected autoencoder (128 independent 256->8->256 per-patch linears
+ sigmoid) on 8 Trainium2 NeuronCores.

Strategy
--------
Feature-parallel: core k owns image-row bands (2k, 2k+1) — each band is 16
image rows x 128 cols = one row of 8 patches — for ALL 2048 samples.  The
host ships x pre-transposed (features on partitions) so the per-patch GEMMs
contract over SBUF partitions with zero on-chip transposes.

Everything rides in fp16 (10-bit mantissa): x ~ N(0,1) never needs bf16's
exponent range, and fp16 matmuls stream at the full bf16 rate on TRN2.  The
whole error budget (~4e-4 per tensor) lands well under the 2e-2 gate, with
HALF the HBM traffic of the fp32 baseline on both input and output — the
kernel is DMA-bound, so bytes are the score.

The autoencoder is linear up to the final sigmoid, so the encode bias
folds into the decode bias on the host: bd' = bd + Wd @ be.  The latent
stage is then a pure PSUM->SBUF fp16 copy, done by the otherwise-idle DVE,
keeping ACT free to do only the 8.4M-element sigmoid stream.

Per band (16 chunks of 512 samples x 2 bands per core):
  encode:  z[64(pw,h), n512] = sum_r We_bd[r].T @ X[r]  (16 fp16 matmuls
           accumulated in PSUM; We_bd is the block-diag arrangement of We)
  latent:  DVE copies z PSUM -> SBUF fp16.
  decode (transposed): out^T[128 f, n512] = Wd_bd[:, fchunk].T @ z
           (1 fp16 matmul per fchunk, K=64); fchunk = one image row r'.
  sigmoid: ACT reads 2-bank decode PSUM -> fp16 out^T tiles (+bd' bias)
           -> DMA to HBM.
The host re-transposes the per-core out^T back to [n, f] and upcasts.
"""

import numpy as np

# problem constants (hardcoded per contract)
H, W, PS = 256, 128, 16
NPH, NPW = H // PS, W // PS      # 16 bands, 8 patches/band
P, D, HID = NPH * NPW, PS * PS, 8
NSMP = 4 * 512                    # 2048 samples
BANDW = PS * W                    # 2048 floats per band per sample
NCORES = 8
BPC = NPH // NCORES               # 2 bands per core
M = NPW * HID                     # 64 latent rows per band
NT = 4                            # sample tiles per band
NW = NSMP // NT                   # 512 samples per tile

_PROG = None
LAST_EXEC_NS = None   # filled when kernel() runs with _trace=True
LAST_RES = None       # full BassKernelResults from the last traced run


def _install_ntff_hook():
    """The agent image's antenv lacks axon_hooks; synthesize it so
    run_bass_kernel_spmd(trace=True) can capture NTFF profiles."""
    import sys, types
    try:
        import antenv.axon_hooks  # noqa: F401
        return
    except ImportError:
        pass
    try:
        from trn_agent_boot.trn_boot import _ntff_profile_via_ctypes
        hook = _ntff_profile_via_ctypes('/opt/axon/libaxon_pjrt.so')
    except Exception:
        hook = None
    import antenv
    mod = types.ModuleType("antenv.axon_hooks")
    mod.get_axon_ntff_profile_hook = lambda: hook
    mod.set_axon_ntff_profile_hook = lambda h: None
    antenv.axon_hooks = mod
    sys.modules["antenv.axon_hooks"] = mod


def _patch_tile_drain():
    """This image's walrus caps instructions at ONE sync wait.  Tile attaches
    one wait per outstanding semaphore to the exit drain and can give body
    instructions several waits.  Split: hoist all but one wait onto fresh
    single-wait NOPs inserted immediately before, on the same engine (engine
    streams are in-order, so this is semantics-preserving)."""
    import concourse.tile as tile
    import bass_rust
    from concourse.vector_clock import ScopedClock

    if getattr(tile.TileContext, "_drain_split_patched", False):
        return

    def patched(self, tick_clock, wait_clock):
        drain_inst = self.nc.sync.drain()
        wait_clock.add_sem_waits(
            drain_inst.ins, ScopedClock({None: tick_clock.global_clock})
        )
        si = drain_inst.ins.sync_info
        w = si.on_wait if si else []
        if len(w) > 1:
            drain_inst.ins.sync_info.on_wait = w[:1]
            for x in w[1:]:
                d2 = self.nc.sync.drain()
                d2.ins.sync_info = bass_rust.SyncInfo(on_wait=[x], on_update=[])
        self.nc.all_engine_barrier()
        assert self.sems is not None
        popped = self.nc._tile_sem_poison_stack.pop()
        assert popped is self._sem_poison
        self.nc.clear_and_free_semaphores(list(self.sems.allocated().values()))
        self.nc.all_engine_barrier()

    tile.TileContext._drain_and_barrier = patched

    from concourse import mybir
    from concourse.tile_scheduler import BassTileLoopBlock, BassTileRelease

    _special = [BassTileLoopBlock, BassTileRelease]
    for nm in ("BassTileCriticalSection", "BassTileBranchHintPlaceholder",
               "TileBranchInst", "BassTileConditionalBlock"):
        cls = getattr(tile, nm, None)
        if cls is not None:
            _special.append(cls)
    _special = tuple(_special)

    orig_lower = tile.TileContext._lower_ordered_insts

    def patched_lower(self, ordered):
        for bb_name in list(ordered.keys()):
            insts = ordered[bb_name]
            if not any(
                i.sync_info is not None and len(i.sync_info.on_wait) > 1
                for i in insts
            ):
                continue
            new = []
            for inst in insts:
                si = inst.sync_info
                if (
                    si is not None
                    and len(si.on_wait) > 1
                    and not isinstance(inst, _special)
                ):
                    waits = list(si.on_wait)
                    for x in waits[:-1]:
                        nop = mybir.InstNoOp(
                            name=self.nc.get_next_instruction_name(),
                            ins=[],
                            outs=[],
                            engine=inst.engine,
                            bass_nofuse=True,
                            sync_info=bass_rust.SyncInfo(on_wait=[x], on_update=[]),
                        )
                        new.append(nop)
                    si.on_wait = waits[-1:]
                new.append(inst)
            ordered[bb_name] = new
        return orig_lower(self, ordered)

    tile.TileContext._lower_ordered_insts = patched_lower
    tile.TileContext._drain_split_patched = True


def _build_program():
    """Build the per-core Bass program (same program for all 8 cores)."""
    global _PROG
    if _PROG is not None:
        return _PROG

    import concourse.bass as bass
    import concourse.tile as tile
    from concourse import mybir

    _patch_tile_drain()

    f32 = mybir.dt.float32
    f16 = mybir.dt.float16
    AFT = mybir.ActivationFunctionType

    nc = bass.Bass("TRN2", target_bir_lowering=False, debug=False)

    # host pre-arranges every tensor partition-major and contiguous
    xt_d = nc.dram_tensor("xt", [BPC, NT, W, PS, NW], f16, kind="ExternalInput").ap()
    we_d = nc.dram_tensor("we", [W, BPC, PS, M], f16, kind="ExternalInput").ap()
    wd_d = nc.dram_tensor("wd", [M, BPC, BANDW], f16, kind="ExternalInput").ap()
    bdv_d = nc.dram_tensor("bdv", [W, BPC, PS], f32, kind="ExternalInput").ap()
    out_d = nc.dram_tensor("out", [BPC, PS, W, NSMP], f16, kind="ExternalOutput").ap()

    with tile.TileContext(nc) as tc:
        with (
            tc.tile_pool(name="singles", bufs=1) as singles,
            tc.tile_pool(name="xp", bufs=4) as xpool,
            tc.tile_pool(name="zhp", bufs=8) as zhpool,
            tc.tile_pool(name="outsb", bufs=3) as opool,
            tc.tile_pool(name="zps", bufs=2, space="PSUM") as zpsum,
            tc.tile_pool(name="ops", bufs=2, space="PSUM") as opsum,
        ):
            # encode weights first: the first matmul needs only these
            we_sb = singles.tile([W, BPC, PS, M], f16)
            nc.sync.dma_start(out=we_sb, in_=we_d)

            x_tiles = {}

            def load_x(b, t):
                X = xpool.tile([W, PS, NW], f16, name="X")
                nc.sync.dma_start(out=X, in_=xt_d[b, t])
                x_tiles[(b, t)] = X

            # band-0 x ahead of the decode-side weights; wd arrives by the
            # time band-0 decode starts (~24us in)
            load_x(0, 0)
            load_x(0, 1)

            wd_sb = singles.tile([M, BPC, BANDW], f16)
            nc.sync.dma_start(out=wd_sb, in_=wd_d)
            bdv_sb = singles.tile([W, BPC, PS], f32)
            nc.sync.dma_start(out=bdv_sb, in_=bdv_d)

            for b in range(BPC):
                zh_tiles = []
                for t in range(NT):
                    if (b, t) not in x_tiles:
                        load_x(b, t)
                    X = x_tiles.pop((b, t))
                    z_ps = zpsum.tile([M, NW], f32, name="z_ps")
                    for r in range(PS):
                        nc.tensor.matmul(
                            z_ps, lhsT=we_sb[:, b, r, :], rhs=X[:, r, :],
                            start=(r == 0), stop=(r == PS - 1),
                        )
                    zh = zhpool.tile([M, NW], f16, name="zh")
                    nc.vector.tensor_copy(zh, z_ps)
                    zh_tiles.append(zh)

                for fc in range(PS):           # one image row r' per chunk
                    o_sb = opool.tile([W, NSMP], f16, name="o_sb")
                    wstat = wd_sb[:, b, fc * W:(fc + 1) * W]
                    for half in range(2):
                        o_ps = opsum.tile([W, 2 * NW], f32, name="o_ps")
                        for j2 in range(2):
                            nc.tensor.matmul(
                                o_ps[:, NW * j2:NW * (j2 + 1)],
                                lhsT=wstat, rhs=zh_tiles[2 * half + j2],
                                start=True, stop=True,
                            )
                        nc.scalar.activation(
                            out=o_sb[:, 2 * NW * half:2 * NW * (half + 1)],
                            in_=o_ps, func=AFT.Sigmoid,
                            bias=bdv_sb[:, b, fc:fc + 1], scale=1.0,
                        )
                    nc.scalar.dma_start(out=out_d[b, fc], in_=o_sb)

    _PROG = nc
    return nc


def _host_prep(x, We, be, Wd, bd):
    """Slice/transpose inputs into per-core maps (pure numpy)."""
    x = np.ascontiguousarray(np.asarray(x, dtype=np.float32)).reshape(NSMP, H * W)
    We = np.asarray(We, dtype=np.float32)
    be = np.asarray(be, dtype=np.float32)
    Wd = np.asarray(Wd, dtype=np.float32)
    bd = np.asarray(bd, dtype=np.float32)

    xT = np.ascontiguousarray(x.T).astype(np.float16)    # [32768, 2048]

    # encode block-diag: wenc[ph, r, 16pw+c, 8pw+h] = We[ph*8+pw, h, r*16+c]
    We6 = We.reshape(NPH, NPW, HID, PS, PS)              # [ph, pw, h, r, c]
    wenc = np.zeros((NPH, PS, W, M), dtype=np.float16)
    for pw in range(NPW):
        wenc[:, :, PS * pw:PS * (pw + 1), HID * pw:HID * (pw + 1)] = (
            We6[:, pw].transpose(0, 2, 3, 1)             # [ph, r, c, h]
        )

    # decode rhs: wdec[ph, 8pw+h, 128r'+16pw+c'] = Wd[ph*8+pw, r'*16+c', h]
    Wd5 = Wd.reshape(NPH, NPW, PS, PS, HID)              # [ph, pw, r', c', h]
    wdec = np.zeros((NPH, M, BANDW), dtype=np.float16)
    wdec_v = wdec.reshape(NPH, NPW, HID, PS, NPW, PS)
    for pw in range(NPW):
        wdec_v[:, pw, :, :, pw, :] = Wd5[:, pw].transpose(0, 3, 1, 2)  # [ph, h, r', c']

    # linear up to the sigmoid -> fold be into the decode bias:
    # bd' = bd + Wd @ be   (per patch)
    bdf = bd + np.einsum('pdh,ph->pd', Wd, be)
    # per-partition for the transposed output: bdv[w', ph, r']
    bd4 = bdf.reshape(NPH, NPW, PS, PS)                  # [ph, pw, r', c']
    bdv = np.ascontiguousarray(
        bd4.transpose(1, 3, 0, 2).reshape(W, NPH, PS))   # [16pw+c', ph, r']

    in_maps = []
    for k in range(NCORES):
        # xt5[b, t, w, r, n] -- per-partition contiguous 16 KiB tiles
        xc = xT[BPC * BANDW * k: BPC * BANDW * (k + 1)]
        xt5 = np.ascontiguousarray(
            xc.reshape(BPC, PS, W, NT, NW).transpose(0, 3, 2, 1, 4))
        in_maps.append({
            "xt": xt5,
            "we": np.ascontiguousarray(
                wenc[BPC * k:BPC * (k + 1)].transpose(2, 0, 1, 3)),
            "wd": np.ascontiguousarray(
                wdec[BPC * k:BPC * (k + 1)].transpose(1, 0, 2)),
            "bdv": np.ascontiguousarray(bdv[:, BPC * k:BPC * (k + 1), :]),
        })
    return in_maps


def kernel(x, We, be, Wd, bd, _trace=False):
    global LAST_EXEC_NS
    from concourse.bass_utils import run_bass_kernel_spmd

    if _trace:
        _install_ntff_hook()

    nc = _build_program()
    in_maps = _host_prep(x, We, be, Wd, bd)
    res = run_bass_kernel_spmd(nc, in_maps, list(range(NCORES)), trace=_trace)
    if _trace:
        LAST_EXEC_NS = res.exec_time_ns
        global LAST_RES
        LAST_RES = res

    # out_k is out^T: [band, r'(=fchunk), w', n]  ->  out[n, band*2048 + 128 r' + w']
    cols = [
        np.asarray(res.results[k]["out"]).reshape(BPC * BANDW, NSMP).T
        for k in range(NCORES)
    ]
    out = np.concatenate(cols, axis=1).astype(np.float32)
    return np.ascontiguousarray(out.reshape(4, 512, H * W))


# revision 5
# speedup vs baseline: 1.6808x; 1.0420x over previous
"""Locally-connected autoencoder (128 independent 256->8->256 per-patch linears
+ sigmoid) on 8 Trainium2 NeuronCores.

Strategy
--------
Feature-parallel: core k owns image-row bands (2k, 2k+1) — each band is 16
image rows x 128 cols = one row of 8 patches — for ALL 2048 samples.  The
host ships x pre-transposed (features on partitions) so the per-patch GEMMs
contract over SBUF partitions with zero on-chip transposes.

Everything rides in fp16 (10-bit mantissa): x ~ N(0,1) never needs bf16's
exponent range, and fp16 matmuls stream at the full bf16 rate on TRN2.  The
whole error budget (~4e-4 per tensor) lands well under the 2e-2 gate, with
HALF the HBM traffic of the fp32 baseline on both input and output — the
kernel is DMA-bound, so bytes are the score.

The autoencoder is linear up to the final sigmoid, so the encode bias
folds into the decode bias on the host: bd' = bd + Wd @ be.  The latent
stage is then a pure PSUM->SBUF fp16 copy, done by the otherwise-idle DVE,
keeping ACT free to do only the 8.4M-element sigmoid stream.

Per band (16 chunks of 512 samples x 2 bands per core):
  encode:  z[64(pw,h), n512] = sum_r We_bd[r].T @ X[r]  (16 fp16 matmuls
           accumulated in PSUM; We_bd is the block-diag arrangement of We)
  latent:  DVE copies z PSUM -> SBUF fp16.
  decode (transposed): out^T[128 f, n512] = Wd_bd[:, fchunk].T @ z
           (1 fp16 matmul per fchunk, K=64); fchunk = one image row r'.
  sigmoid: ACT reads 2-bank decode PSUM -> fp16 out^T tiles (+bd' bias)
           -> DMA to HBM.
The host re-transposes the per-core out^T back to [n, f] and upcasts.
"""

import numpy as np

# problem constants (hardcoded per contract)
H, W, PS = 256, 128, 16
NPH, NPW = H // PS, W // PS      # 16 bands, 8 patches/band
P, D, HID = NPH * NPW, PS * PS, 8
NSMP = 4 * 512                    # 2048 samples
BANDW = PS * W                    # 2048 floats per band per sample
NCORES = 8
BPC = NPH // NCORES               # 2 bands per core
M = NPW * HID                     # 64 latent rows per band
NT = 4                            # sample tiles per band
NW = NSMP // NT                   # 512 samples per tile

_PROG = None
LAST_EXEC_NS = None   # filled when kernel() runs with _trace=True
LAST_RES = None       # full BassKernelResults from the last traced run


def _install_ntff_hook():
    """The agent image's antenv lacks axon_hooks; synthesize it so
    run_bass_kernel_spmd(trace=True) can capture NTFF profiles."""
    import sys, types
    try:
        import antenv.axon_hooks  # noqa: F401
        return
    except ImportError:
        pass
    try:
        from trn_agent_boot.trn_boot import _ntff_profile_via_ctypes
        hook = _ntff_profile_via_ctypes('/opt/axon/libaxon_pjrt.so')
    except Exception:
        hook = None
    import antenv
    mod = types.ModuleType("antenv.axon_hooks")
    mod.get_axon_ntff_profile_hook = lambda: hook
    mod.set_axon_ntff_profile_hook = lambda h: None
    antenv.axon_hooks = mod
    sys.modules["antenv.axon_hooks"] = mod


def _patch_tile_drain():
    """This image's walrus caps instructions at ONE sync wait.  Tile attaches
    one wait per outstanding semaphore to the exit drain and can give body
    instructions several waits.  Split: hoist all but one wait onto fresh
    single-wait NOPs inserted immediately before, on the same engine (engine
    streams are in-order, so this is semantics-preserving)."""
    import concourse.tile as tile
    import bass_rust
    from concourse.vector_clock import ScopedClock

    if getattr(tile.TileContext, "_drain_split_patched", False):
        return

    def patched(self, tick_clock, wait_clock):
        drain_inst = self.nc.sync.drain()
        wait_clock.add_sem_waits(
            drain_inst.ins, ScopedClock({None: tick_clock.global_clock})
        )
        si = drain_inst.ins.sync_info
        w = si.on_wait if si else []
        if len(w) > 1:
            drain_inst.ins.sync_info.on_wait = w[:1]
            for x in w[1:]:
                d2 = self.nc.sync.drain()
                d2.ins.sync_info = bass_rust.SyncInfo(on_wait=[x], on_update=[])
        self.nc.all_engine_barrier()
        assert self.sems is not None
        popped = self.nc._tile_sem_poison_stack.pop()
        assert popped is self._sem_poison
        self.nc.clear_and_free_semaphores(list(self.sems.allocated().values()))
        self.nc.all_engine_barrier()

    tile.TileContext._drain_and_barrier = patched

    from concourse import mybir
    from concourse.tile_scheduler import BassTileLoopBlock, BassTileRelease

    _special = [BassTileLoopBlock, BassTileRelease]
    for nm in ("BassTileCriticalSection", "BassTileBranchHintPlaceholder",
               "TileBranchInst", "BassTileConditionalBlock"):
        cls = getattr(tile, nm, None)
        if cls is not None:
            _special.append(cls)
    _special = tuple(_special)

    orig_lower = tile.TileContext._lower_ordered_insts

    def patched_lower(self, ordered):
        for bb_name in list(ordered.keys()):
            insts = ordered[bb_name]
            if not any(
                i.sync_info is not None and len(i.sync_info.on_wait) > 1
                for i in insts
            ):
                continue
            new = []
            for inst in insts:
                si = inst.sync_info
                if (
                    si is not None
                    and len(si.on_wait) > 1
                    and not isinstance(inst, _special)
                ):
                    waits = list(si.on_wait)
                    for x in waits[:-1]:
                        nop = mybir.InstNoOp(
                            name=self.nc.get_next_instruction_name(),
                            ins=[],
                            outs=[],
                            engine=inst.engine,
                            bass_nofuse=True,
                            sync_info=bass_rust.SyncInfo(on_wait=[x], on_update=[]),
                        )
                        new.append(nop)
                    si.on_wait = waits[-1:]
                new.append(inst)
            ordered[bb_name] = new
        return orig_lower(self, ordered)

    tile.TileContext._lower_ordered_insts = patched_lower
    tile.TileContext._drain_split_patched = True


def _build_program():
    """Build the per-core Bass program (same program for all 8 cores)."""
    global _PROG
    if _PROG is not None:
        return _PROG

    import concourse.bass as bass
    import concourse.tile as tile
    from concourse import mybir

    _patch_tile_drain()

    f32 = mybir.dt.float32
    f16 = mybir.dt.float16
    AFT = mybir.ActivationFunctionType

    nc = bass.Bass("TRN2", target_bir_lowering=False, debug=False)

    # host pre-arranges every tensor partition-major and contiguous
    xt_d = nc.dram_tensor("xt", [BPC, NT, W, PS, NW], f16, kind="ExternalInput").ap()
    we_d = nc.dram_tensor("we", [W, BPC, PS, M], f16, kind="ExternalInput").ap()
    wd_d = nc.dram_tensor("wd", [M, BPC, BANDW], f16, kind="ExternalInput").ap()
    bdv_d = nc.dram_tensor("bdv", [W, BPC, PS], f32, kind="ExternalInput").ap()
    out_d = nc.dram_tensor("out", [BPC, PS, W, NSMP], f16, kind="ExternalOutput").ap()

    with tile.TileContext(nc) as tc:
        with (
            tc.tile_pool(name="singles", bufs=1) as singles,
            tc.tile_pool(name="xp", bufs=2 * NT) as xpool,
            tc.tile_pool(name="zhp", bufs=6) as zhpool,
            tc.tile_pool(name="outsb", bufs=22) as opool,
            tc.tile_pool(name="zps", bufs=2, space="PSUM") as zpsum,
            tc.tile_pool(name="ops", bufs=3, space="PSUM") as opsum,
        ):
            # encode weights first: the first matmul needs only these
            we_sb = singles.tile([W, BPC, PS, M], f16)
            nc.sync.dma_start(out=we_sb, in_=we_d)

            x_tiles = {}

            def load_x(b, t):
                X = xpool.tile([W, PS, NW], f16, name="X")
                nc.sync.dma_start(out=X, in_=xt_d[b, t])
                x_tiles[(b, t)] = X

            # all x up-front (pool holds all 8 tiles -> the input stream
            # never stalls); wd slots in early enough for the first decode
            load_x(0, 0)
            load_x(0, 1)

            wd_sb = singles.tile([M, BPC, BANDW], f16)
            nc.sync.dma_start(out=wd_sb, in_=wd_d)
            bdv_sb = singles.tile([W, BPC, PS], f32)
            nc.sync.dma_start(out=bdv_sb, in_=bdv_d)

            for b in range(BPC):
                for t in range(NT):
                    if (b, t) not in x_tiles:
                        load_x(b, t)

            # half-band pipeline: g = (band, n-half).  Encode of half-band
            # g+1 is interleaved into the decode fc-loop of g (2 matmuls per
            # fc) so the PE stream stays dense and ACT never waits at a
            # half-band boundary.
            halves = [(b, h) for b in range(BPC) for h in range(2)]

            def encode_mm(g, r):
                """Issue encode matmul row r for both chunks of half-band g."""
                b, h = g
                for c in range(2):
                    t = 2 * h + c
                    nc.tensor.matmul(
                        z_ps_cur[c], lhsT=we_sb[:, b, r, :],
                        rhs=x_tiles[(b, t)][:, r, :],
                        start=(r == 0), stop=(r == PS - 1),
                    )

            def encode_finish(g):
                """PSUM -> SBUF fp16 latent copies for half-band g."""
                b, h = g
                zh_pair = []
                for c in range(2):
                    zh = zhpool.tile([M, NW], f16, name="zh")
                    nc.vector.tensor_copy(zh, z_ps_cur[c])
                    zh_pair.append(zh)
                return zh_pair

            def encode_start():
                return [zpsum.tile([M, NW], f32, name="z_ps") for _ in range(2)]

            # prologue: encode half-band 0 densely
            z_ps_cur = encode_start()
            for r in range(PS):
                encode_mm(halves[0], r)
            zh_cur = encode_finish(halves[0])

            for gi, g in enumerate(halves):
                b, h = g
                nxt = halves[gi + 1] if gi + 1 < len(halves) else None
                if nxt is not None:
                    z_ps_cur = encode_start()
                out_eng = nc.scalar if gi % 2 == 0 else nc.sync
                zh_pair = zh_cur
                for fc in range(PS):
                    o_ps = opsum.tile([W, 2 * NW], f32, name="o_ps")
                    wstat = wd_sb[:, b, fc * W:(fc + 1) * W]
                    for c in range(2):
                        nc.tensor.matmul(
                            o_ps[:, NW * c:NW * (c + 1)],
                            lhsT=wstat, rhs=zh_pair[c],
                            start=True, stop=True,
                        )
                    if nxt is not None:
                        encode_mm(nxt, fc)
                    o_sb = opool.tile([W, 2 * NW], f16, name="o_sb")
                    nc.scalar.activation(
                        out=o_sb, in_=o_ps, func=AFT.Sigmoid,
                        bias=bdv_sb[:, b, fc:fc + 1], scale=1.0,
                    )
                    out_eng.dma_start(
                        out=out_d[b, fc, :, 2 * NW * h:2 * NW * (h + 1)],
                        in_=o_sb,
                    )
                if nxt is not None:
                    zh_cur = encode_finish(nxt)

    _PROG = nc
    return nc


def _host_prep(x, We, be, Wd, bd):
    """Slice/transpose inputs into per-core maps (pure numpy)."""
    x = np.ascontiguousarray(np.asarray(x, dtype=np.float32)).reshape(NSMP, H * W)
    We = np.asarray(We, dtype=np.float32)
    be = np.asarray(be, dtype=np.float32)
    Wd = np.asarray(Wd, dtype=np.float32)
    bd = np.asarray(bd, dtype=np.float32)

    xT = np.ascontiguousarray(x.T).astype(np.float16)    # [32768, 2048]

    # encode block-diag: wenc[ph, r, 16pw+c, 8pw+h] = We[ph*8+pw, h, r*16+c]
    We6 = We.reshape(NPH, NPW, HID, PS, PS)              # [ph, pw, h, r, c]
    wenc = np.zeros((NPH, PS, W, M), dtype=np.float16)
    for pw in range(NPW):
        wenc[:, :, PS * pw:PS * (pw + 1), HID * pw:HID * (pw + 1)] = (
            We6[:, pw].transpose(0, 2, 3, 1)             # [ph, r, c, h]
        )

    # decode rhs: wdec[ph, 8pw+h, 128r'+16pw+c'] = Wd[ph*8+pw, r'*16+c', h]
    Wd5 = Wd.reshape(NPH, NPW, PS, PS, HID)              # [ph, pw, r', c', h]
    wdec = np.zeros((NPH, M, BANDW), dtype=np.float16)
    wdec_v = wdec.reshape(NPH, NPW, HID, PS, NPW, PS)
    for pw in range(NPW):
        wdec_v[:, pw, :, :, pw, :] = Wd5[:, pw].transpose(0, 3, 1, 2)  # [ph, h, r', c']

    # linear up to the sigmoid -> fold be into the decode bias:
    # bd' = bd + Wd @ be   (per patch)
    bdf = bd + np.einsum('pdh,ph->pd', Wd, be)
    # per-partition for the transposed output: bdv[w', ph, r']
    bd4 = bdf.reshape(NPH, NPW, PS, PS)                  # [ph, pw, r', c']
    bdv = np.ascontiguousarray(
        bd4.transpose(1, 3, 0, 2).reshape(W, NPH, PS))   # [16pw+c', ph, r']

    in_maps = []
    for k in range(NCORES):
        # xt5[b, t, w, r, n] -- per-partition contiguous 16 KiB tiles
        xc = xT[BPC * BANDW * k: BPC * BANDW * (k + 1)]
        xt5 = np.ascontiguousarray(
            xc.reshape(BPC, PS, W, NT, NW).transpose(0, 3, 2, 1, 4))
        in_maps.append({
            "xt": xt5,
            "we": np.ascontiguousarray(
                wenc[BPC * k:BPC * (k + 1)].transpose(2, 0, 1, 3)),
            "wd": np.ascontiguousarray(
                wdec[BPC * k:BPC * (k + 1)].transpose(1, 0, 2)),
            "bdv": np.ascontiguousarray(bdv[:, BPC * k:BPC * (k + 1), :]),
        })
    return in_maps


def kernel(x, We, be, Wd, bd, _trace=False):
    global LAST_EXEC_NS
    from concourse.bass_utils import run_bass_kernel_spmd

    if _trace:
        _install_ntff_hook()

    nc = _build_program()
    in_maps = _host_prep(x, We, be, Wd, bd)
    res = run_bass_kernel_spmd(nc, in_maps, list(range(NCORES)), trace=_trace)
    if _trace:
        LAST_EXEC_NS = res.exec_time_ns
        global LAST_RES
        LAST_RES = res

    # out_k is out^T: [band, r'(=fchunk), w', n]  ->  out[n, band*2048 + 128 r' + w']
    cols = [
        np.asarray(res.results[k]["out"]).reshape(BPC * BANDW, NSMP).T
        for k in range(NCORES)
    ]
    out = np.concatenate(cols, axis=1).astype(np.float32)
    return np.ascontiguousarray(out.reshape(4, 512, H * W))


# revision 6
# speedup vs baseline: 1.9433x; 1.1562x over previous
"""Locally-connected autoencoder (128 independent 256->8->256 per-patch linears
+ sigmoid) on 8 Trainium2 NeuronCores.

Strategy
--------
Feature-parallel: core k owns image-row bands (2k, 2k+1) — each band is 16
image rows x 128 cols = one row of 8 patches — for ALL 2048 samples.  The
host ships x pre-transposed (features on partitions) so the per-patch GEMMs
contract over SBUF partitions with zero on-chip transposes.

Everything rides in fp16 (10-bit mantissa): x ~ N(0,1) never needs bf16's
exponent range, and fp16 matmuls stream at the full bf16 rate on TRN2.  The
whole error budget (~4e-4 per tensor) lands well under the 2e-2 gate, with
HALF the HBM traffic of the fp32 baseline on both input and output — the
kernel is DMA-bound, so bytes are the score.

The autoencoder is linear up to the final sigmoid, so the encode bias
folds into the decode bias on the host: bd' = bd + Wd @ be.  The latent
stage is then a pure PSUM->SBUF fp16 copy, done by the otherwise-idle DVE,
keeping ACT free to do only the 8.4M-element sigmoid stream.

Per band (16 chunks of 512 samples x 2 bands per core):
  encode:  z[64(pw,h), n512] = sum_r We_bd[r].T @ X[r]  (16 fp16 matmuls
           accumulated in PSUM; We_bd is the block-diag arrangement of We)
  latent:  DVE copies z PSUM -> SBUF fp16.
  decode (transposed): out^T[128 f, n512] = Wd_bd[:, fchunk].T @ z
           (1 fp16 matmul per fchunk, K=64); fchunk = one image row r'.
  sigmoid: ACT reads 2-bank decode PSUM -> fp16 out^T tiles (+bd' bias)
           -> DMA to HBM.
The host re-transposes the per-core out^T back to [n, f] and upcasts.
"""

import numpy as np

# problem constants (hardcoded per contract)
H, W, PS = 256, 128, 16
NPH, NPW = H // PS, W // PS      # 16 bands, 8 patches/band
P, D, HID = NPH * NPW, PS * PS, 8
NSMP = 4 * 512                    # 2048 samples
BANDW = PS * W                    # 2048 floats per band per sample
NCORES = 8
BPC = NPH // NCORES               # 2 bands per core
M = NPW * HID                     # 64 latent rows per band
NT = 4                            # sample tiles per band
NW = NSMP // NT                   # 512 samples per tile

_PROG = None
LAST_EXEC_NS = None   # filled when kernel() runs with _trace=True
LAST_RES = None       # full BassKernelResults from the last traced run


def _install_ntff_hook():
    """The agent image's antenv lacks axon_hooks; synthesize it so
    run_bass_kernel_spmd(trace=True) can capture NTFF profiles."""
    import sys, types
    try:
        import antenv.axon_hooks  # noqa: F401
        return
    except ImportError:
        pass
    try:
        from trn_agent_boot.trn_boot import _ntff_profile_via_ctypes
        hook = _ntff_profile_via_ctypes('/opt/axon/libaxon_pjrt.so')
    except Exception:
        hook = None
    import antenv
    mod = types.ModuleType("antenv.axon_hooks")
    mod.get_axon_ntff_profile_hook = lambda: hook
    mod.set_axon_ntff_profile_hook = lambda h: None
    antenv.axon_hooks = mod
    sys.modules["antenv.axon_hooks"] = mod


def _patch_tile_drain():
    """This image's walrus caps instructions at ONE sync wait.  Tile attaches
    one wait per outstanding semaphore to the exit drain and can give body
    instructions several waits.  Split: hoist all but one wait onto fresh
    single-wait NOPs inserted immediately before, on the same engine (engine
    streams are in-order, so this is semantics-preserving)."""
    import concourse.tile as tile
    import bass_rust
    from concourse.vector_clock import ScopedClock

    if getattr(tile.TileContext, "_drain_split_patched", False):
        return

    def patched(self, tick_clock, wait_clock):
        drain_inst = self.nc.sync.drain()
        wait_clock.add_sem_waits(
            drain_inst.ins, ScopedClock({None: tick_clock.global_clock})
        )
        si = drain_inst.ins.sync_info
        w = si.on_wait if si else []
        if len(w) > 1:
            drain_inst.ins.sync_info.on_wait = w[:1]
            for x in w[1:]:
                d2 = self.nc.sync.drain()
                d2.ins.sync_info = bass_rust.SyncInfo(on_wait=[x], on_update=[])
        self.nc.all_engine_barrier()
        assert self.sems is not None
        popped = self.nc._tile_sem_poison_stack.pop()
        assert popped is self._sem_poison
        self.nc.clear_and_free_semaphores(list(self.sems.allocated().values()))
        self.nc.all_engine_barrier()

    tile.TileContext._drain_and_barrier = patched

    from concourse import mybir
    from concourse.tile_scheduler import BassTileLoopBlock, BassTileRelease

    _special = [BassTileLoopBlock, BassTileRelease]
    for nm in ("BassTileCriticalSection", "BassTileBranchHintPlaceholder",
               "TileBranchInst", "BassTileConditionalBlock"):
        cls = getattr(tile, nm, None)
        if cls is not None:
            _special.append(cls)
    _special = tuple(_special)

    orig_lower = tile.TileContext._lower_ordered_insts

    def patched_lower(self, ordered):
        for bb_name in list(ordered.keys()):
            insts = ordered[bb_name]
            if not any(
                i.sync_info is not None and len(i.sync_info.on_wait) > 1
                for i in insts
            ):
                continue
            new = []
            for inst in insts:
                si = inst.sync_info
                if (
                    si is not None
                    and len(si.on_wait) > 1
                    and not isinstance(inst, _special)
                ):
                    waits = list(si.on_wait)
                    for x in waits[:-1]:
                        nop = mybir.InstNoOp(
                            name=self.nc.get_next_instruction_name(),
                            ins=[],
                            outs=[],
                            engine=inst.engine,
                            bass_nofuse=True,
                            sync_info=bass_rust.SyncInfo(on_wait=[x], on_update=[]),
                        )
                        new.append(nop)
                    si.on_wait = waits[-1:]
                new.append(inst)
            ordered[bb_name] = new
        return orig_lower(self, ordered)

    tile.TileContext._lower_ordered_insts = patched_lower
    tile.TileContext._drain_split_patched = True


def _build_program():
    """Build the per-core Bass program (same program for all 8 cores)."""
    global _PROG
    if _PROG is not None:
        return _PROG

    import concourse.bass as bass
    import concourse.tile as tile
    from concourse import mybir

    _patch_tile_drain()

    f32 = mybir.dt.float32
    f16 = mybir.dt.float16
    AFT = mybir.ActivationFunctionType

    nc = bass.Bass("TRN2", target_bir_lowering=False, debug=False)

    # host pre-arranges every tensor partition-major and contiguous
    xt_d = nc.dram_tensor("xt", [BPC, NT, W, PS, NW], f16, kind="ExternalInput").ap()
    we_d = nc.dram_tensor("we", [W, BPC, PS, M], f16, kind="ExternalInput").ap()
    wd_d = nc.dram_tensor("wd", [M, BPC, BANDW], f16, kind="ExternalInput").ap()
    bdv_d = nc.dram_tensor("bdv", [W, BPC, PS], f32, kind="ExternalInput").ap()
    out_d = nc.dram_tensor("out", [BPC, PS, W, NSMP], f16, kind="ExternalOutput").ap()

    with tile.TileContext(nc) as tc:
        with (
            tc.tile_pool(name="singles", bufs=1) as singles,
            tc.tile_pool(name="xp", bufs=2 * NT) as xpool,
            tc.tile_pool(name="zhp", bufs=6) as zhpool,
            tc.tile_pool(name="outsb", bufs=22) as opool,
            tc.tile_pool(name="zps", bufs=2, space="PSUM") as zpsum,
            tc.tile_pool(name="ops", bufs=3, space="PSUM") as opsum,
        ):
            # encode weights first: the first matmul needs only these
            we_sb = singles.tile([W, BPC, PS, M], f16)
            nc.sync.dma_start(out=we_sb, in_=we_d)

            x_tiles = {}

            def load_x(b, t):
                X = xpool.tile([W, PS, NW], f16, name="X")
                nc.sync.dma_start(out=X, in_=xt_d[b, t])
                x_tiles[(b, t)] = X

            # all x up-front (pool holds all 8 tiles -> the input stream
            # never stalls); wd slots in early enough for the first decode
            load_x(0, 0)
            load_x(0, 1)

            wd_sb = singles.tile([M, BPC, BANDW], f16)
            nc.sync.dma_start(out=wd_sb, in_=wd_d)
            bdv_sb = singles.tile([W, BPC, PS], f32)
            nc.sync.dma_start(out=bdv_sb, in_=bdv_d)

            for b in range(BPC):
                for t in range(NT):
                    if (b, t) not in x_tiles:
                        load_x(b, t)

            # half-band pipeline: g = (band, n-half).  Encode of half-band
            # g+1 is interleaved into the decode fc-loop of g (2 matmuls per
            # fc) so the PE stream stays dense and ACT never waits at a
            # half-band boundary.
            halves = [(b, h) for b in range(BPC) for h in range(2)]

            def encode_mm(g, r):
                """Issue encode matmul row r for both chunks of half-band g."""
                b, h = g
                for c in range(2):
                    t = 2 * h + c
                    nc.tensor.matmul(
                        z_ps_cur[c], lhsT=we_sb[:, b, r, :],
                        rhs=x_tiles[(b, t)][:, r, :],
                        start=(r == 0), stop=(r == PS - 1),
                    )

            def encode_finish(g):
                """PSUM -> SBUF fp16 latent copies for half-band g."""
                b, h = g
                zh_pair = []
                for c in range(2):
                    zh = zhpool.tile([M, NW], f16, name="zh")
                    nc.vector.tensor_copy(zh, z_ps_cur[c])
                    zh_pair.append(zh)
                return zh_pair

            def encode_start():
                return [zpsum.tile([M, NW], f32, name="z_ps") for _ in range(2)]

            # prologue: encode half-band 0 densely
            z_ps_cur = encode_start()
            for r in range(PS):
                encode_mm(halves[0], r)
            zh_cur = encode_finish(halves[0])

            for gi, g in enumerate(halves):
                b, h = g
                nxt = halves[gi + 1] if gi + 1 < len(halves) else None
                if nxt is not None:
                    z_ps_cur = encode_start()
                # keep ACT free of DMA issues: even half-bands ride the idle
                # GpSimd's SWDGE, odd ones the sync ring (inputs done by then)
                out_eng = nc.gpsimd if gi % 2 == 0 else nc.sync
                zh_pair = zh_cur
                for fc in range(PS):
                    o_ps = opsum.tile([W, 2 * NW], f32, name="o_ps")
                    wstat = wd_sb[:, b, fc * W:(fc + 1) * W]
                    for c in range(2):
                        nc.tensor.matmul(
                            o_ps[:, NW * c:NW * (c + 1)],
                            lhsT=wstat, rhs=zh_pair[c],
                            start=True, stop=True,
                        )
                    if nxt is not None:
                        encode_mm(nxt, fc)
                    o_sb = opool.tile([W, 2 * NW], f16, name="o_sb")
                    nc.scalar.activation(
                        out=o_sb, in_=o_ps, func=AFT.Sigmoid,
                        bias=bdv_sb[:, b, fc:fc + 1], scale=1.0,
                    )
                    out_eng.dma_start(
                        out=out_d[b, fc, :, 2 * NW * h:2 * NW * (h + 1)],
                        in_=o_sb,
                    )
                if nxt is not None:
                    zh_cur = encode_finish(nxt)

    _PROG = nc
    return nc


def _host_prep(x, We, be, Wd, bd):
    """Slice/transpose inputs into per-core maps (pure numpy)."""
    x = np.ascontiguousarray(np.asarray(x, dtype=np.float32)).reshape(NSMP, H * W)
    We = np.asarray(We, dtype=np.float32)
    be = np.asarray(be, dtype=np.float32)
    Wd = np.asarray(Wd, dtype=np.float32)
    bd = np.asarray(bd, dtype=np.float32)

    xT = np.ascontiguousarray(x.T).astype(np.float16)    # [32768, 2048]

    # encode block-diag: wenc[ph, r, 16pw+c, 8pw+h] = We[ph*8+pw, h, r*16+c]
    We6 = We.reshape(NPH, NPW, HID, PS, PS)              # [ph, pw, h, r, c]
    wenc = np.zeros((NPH, PS, W, M), dtype=np.float16)
    for pw in range(NPW):
        wenc[:, :, PS * pw:PS * (pw + 1), HID * pw:HID * (pw + 1)] = (
            We6[:, pw].transpose(0, 2, 3, 1)             # [ph, r, c, h]
        )

    # decode rhs: wdec[ph, 8pw+h, 128r'+16pw+c'] = Wd[ph*8+pw, r'*16+c', h]
    Wd5 = Wd.reshape(NPH, NPW, PS, PS, HID)              # [ph, pw, r', c', h]
    wdec = np.zeros((NPH, M, BANDW), dtype=np.float16)
    wdec_v = wdec.reshape(NPH, NPW, HID, PS, NPW, PS)
    for pw in range(NPW):
        wdec_v[:, pw, :, :, pw, :] = Wd5[:, pw].transpose(0, 3, 1, 2)  # [ph, h, r', c']

    # linear up to the sigmoid -> fold be into the decode bias:
    # bd' = bd + Wd @ be   (per patch)
    bdf = bd + np.einsum('pdh,ph->pd', Wd, be)
    # per-partition for the transposed output: bdv[w', ph, r']
    bd4 = bdf.reshape(NPH, NPW, PS, PS)                  # [ph, pw, r', c']
    bdv = np.ascontiguousarray(
        bd4.transpose(1, 3, 0, 2).reshape(W, NPH, PS))   # [16pw+c', ph, r']

    in_maps = []
    for k in range(NCORES):
        # xt5[b, t, w, r, n] -- per-partition contiguous 16 KiB tiles
        xc = xT[BPC * BANDW * k: BPC * BANDW * (k + 1)]
        xt5 = np.ascontiguousarray(
            xc.reshape(BPC, PS, W, NT, NW).transpose(0, 3, 2, 1, 4))
        in_maps.append({
            "xt": xt5,
            "we": np.ascontiguousarray(
                wenc[BPC * k:BPC * (k + 1)].transpose(2, 0, 1, 3)),
            "wd": np.ascontiguousarray(
                wdec[BPC * k:BPC * (k + 1)].transpose(1, 0, 2)),
            "bdv": np.ascontiguousarray(bdv[:, BPC * k:BPC * (k + 1), :]),
        })
    return in_maps


def kernel(x, We, be, Wd, bd, _trace=False):
    global LAST_EXEC_NS
    from concourse.bass_utils import run_bass_kernel_spmd

    if _trace:
        _install_ntff_hook()

    nc = _build_program()
    in_maps = _host_prep(x, We, be, Wd, bd)
    res = run_bass_kernel_spmd(nc, in_maps, list(range(NCORES)), trace=_trace)
    if _trace:
        LAST_EXEC_NS = res.exec_time_ns
        global LAST_RES
        LAST_RES = res

    # out_k is out^T: [band, r'(=fchunk), w', n]  ->  out[n, band*2048 + 128 r' + w']
    cols = [
        np.asarray(res.results[k]["out"]).reshape(BPC * BANDW, NSMP).T
        for k in range(NCORES)
    ]
    out = np.concatenate(cols, axis=1).astype(np.float32)
    return np.ascontiguousarray(out.reshape(4, 512, H * W))
